# revision 22
# baseline (speedup 1.0000x reference)
"""CodeGen attention on 8 Trainium2 NeuronCores (Bass/Tile), fp16 datapath.

Sharding: tensor-parallel over the 4 CodeGen mp head-groups x data-parallel
over batch 2. Core c = dp*4 + tp handles batch dp, head group tp (4 heads).

Per-core pipeline (all matmul inputs fp16, PSUM accumulation fp32):
  QKV (per 1024-wide s slice): V in natural [s, d] layout (x^T stationary,
      Wv moving), then Q^T/K^T (W stationary, x^T moving) with rotary
      fused on the first 64 rows of each head.  Weights arrive host
      pre-swizzled so every DMA runs fat contiguous per-partition
      segments.  The second s-slice's Q/K tiles are emitted head by head,
      interleaved with that head's attention, so the per-head AllGather
      collectives start as early as possible and overlap compute.
  Attention: scores^T tiles, softmax without max-subtraction (logits are
      O(5)), column sums via an ones-vector matmul, PV accumulated in
      PSUM, late normalize by 1/rowsum broadcast through a K=1 matmul.
  Out-projection: contraction tiles stream per head so accumulation
      starts as soon as AllGather h lands.  q/k/og scratch is split into
      per-head DRAM tensors to keep cross-phase dependencies fine-grained.

Host assembles the [B, S, D] output from per-core [S, D/4] column shards.
"""

import numpy as np

B, S, D = 2, 2048, 4096
N_HEAD = 16
HD = 256
MP = 4
ROT = 64
LOCAL = D // MP            # 1024 (= 4 heads * 256)
DT = D // 128              # 32 contraction tiles
N_CORES = 8
H_LOC = N_HEAD // MP       # 4 heads per core

_CACHE = {}
_WITH_COLL = True   # timing-probe flag: False replaces AllGather with DMA copies


def _emit_body(nc, tc, tens, psp, rep):
    """One full pipeline pass (scratch tensors shared across reps)."""
    import concourse.tile as tile  # noqa: F401
    from concourse import mybir

    f32 = mybir.dt.float32
    f32r = mybir.dt.float32r
    f16 = mybir.dt.float16
    EXP = mybir.ActivationFunctionType.Exp

    (xt2, wqk, wv2, wo2, cost, sint, masks, y,
     qT_hs, kT_hs, v_d01, v_d23, og_ins, og_outs,
     rt_sb, ones_sb, onesr_sb) = tens

    def emit_v_vn(ss, vn, xr, wvp, vcp):
        wvt = wvp.tile([128, DT, 512], f16, name="wvt", tag="wvt")
        nc.sync.dma_start(wvt[:], wv2.ap()[:, vn])
        vd = v_d01 if vn == 0 else v_d23
        for sm in range(8):
            pv = psp.tile([128, 512], f32, name="pv", tag="ps")
            for dt in range(DT):
                nc.tensor.matmul(
                    pv[:], xr[:, dt, sm * 128:(sm + 1) * 128],
                    wvt[:, dt, :], start=(dt == 0), stop=(dt == DT - 1))
            vc = vcp.tile([128, 512], f16, name="vc", tag="vc")
            nc.vector.tensor_copy(vc[:], pv[:])
            st = ss * 8 + sm
            for hh in range(2):
                nc.sync.dma_start(
                    vd[:, hh, st, :], vc[:, hh * 256:(hh + 1) * 256])

    def emit_qk_tile(m, ss, xr, wmp, sqp, t1p, t2p, cost_sb, sint_sb,
                     direct=None):
        """direct: a [128, 2, S] SBUF tile — ss1 results land there
        straight from PSUM (no DRAM round-trip)."""
        wm = wmp.tile([128, DT, 128], f16, name="wm", tag="wm")
        nc.sync.dma_start(wm[:], wqk.ap()[:, m])
        dd = m % 2
        for n in range(2):
            ps = psp.tile([128, 512], f32, name="ps", tag="ps")
            for dt in range(DT):
                nc.tensor.matmul(
                    ps[:], wm[:, dt, :], xr[:, dt, n * 512:(n + 1) * 512],
                    start=(dt == 0), stop=(dt == DT - 1))
            sg = ss * 1024 + n * 512
            if direct is None:
                sq = sqp.tile([128, 512], f16, name="sq", tag="sq")
                tgt, tgt_rot = sq[:], sq[0:ROT, :]
            else:
                tgt = direct[:, dd, sg:sg + 512]
                tgt_rot = direct[0:ROT, dd, sg:sg + 512]
            nc.vector.tensor_copy(tgt, ps[:])
            if m % 2 == 0:
                # rows 0:64 are the rotary dims of a head
                rp = psp.tile([128, 512], f32, name="rp", tag="ps")
                nc.tensor.matmul(rp[0:ROT, :], rt_sb[:],
                                 tgt_rot, start=True, stop=True)
                t1 = t1p.tile([ROT, 512], f32, name="t1", tag="t1")
                nc.vector.tensor_mul(t1[:], ps[0:ROT, :],
                                     cost_sb[:, sg:sg + 512])
                t2 = t2p.tile([ROT, 512], f32, name="t2", tag="t2")
                nc.vector.tensor_mul(t2[:], rp[0:ROT, :],
                                     sint_sb[:, sg:sg + 512])
                nc.vector.tensor_add(tgt_rot, t1[:], t2[:])
            if direct is None:
                dest = qT_hs[m // 2] if m < 8 else kT_hs[(m - 8) // 2]
                nc.sync.dma_start(
                    dest[dd * 128:(dd + 1) * 128, sg:sg + 512], tgt)

    def emit_attention(h, pools, qt, kt):
        qtp, ktp, vtp, etp, etmp, otp, rbp, rip, masks_sb = pools
        vt = vtp.tile([128, S // 128, HD], f16, name="vt", tag="vt")
        vd = v_d01 if h < 2 else v_d23
        nc.sync.dma_start(vt[:], vd[:, h % 2])
        ot = otp.tile([128, 2, S], f16, name="ot", tag="ot")

        for qn in range(4):
            nk = (qn + 1) * 4
            q0 = qn * 512
            rs = psp.tile([1, 512], f32, name="rs", tag="ps")
            ov = [psp.tile([128, 512], f32, name=f"ov{dm}", tag="ps")
                  for dm in range(2)]
            for ki in range(nk):
                sp = psp.tile([128, 512], f32, name="sp", tag="ps")
                for dd in range(2):
                    nc.tensor.matmul(
                        sp[:], kt[:, dd, ki * 128:(ki + 1) * 128],
                        qt[:, dd, q0:q0 + 512],
                        start=(dd == 0), stop=(dd == 1))
                et = etp.tile([128, 512], f16, name="et", tag="et")
                if ki >= qn * 4:
                    etm = etmp.tile([128, 512], f16, name="etm", tag="etm")
                    nc.scalar.activation(etm[:], sp[:], EXP,
                                         bias=0.0, scale=1.0 / 16.0)
                    nc.vector.tensor_mul(et[:], etm[:],
                                         masks_sb[:, ki - qn * 4, :])
                else:
                    nc.scalar.activation(et[:], sp[:], EXP,
                                         bias=0.0, scale=1.0 / 16.0)
                nc.tensor.matmul(rs[:], ones_sb[:], et[:],
                                 start=(ki == 0), stop=(ki == nk - 1))
                for dm in range(2):
                    nc.tensor.matmul(
                        ov[dm][:], vt[:, ki, dm * 128:(dm + 1) * 128],
                        et[:], start=(ki == 0), stop=(ki == nk - 1))
            rinv = rip.tile([1, 512], f32r, name="rinv", tag="rinv")
            # f32r is bit-identical to f32 here; only the matmul
            # datapath reads it differently.
            with nc.allow_low_precision(reason="f32r == f32 bits"):
                nc.vector.reciprocal(rinv[:], rs[:])
            rb = psp.tile([128, 512], f32, name="rb", tag="ps")
            nc.tensor.matmul(rb[:], onesr_sb[:], rinv[:],
                             start=True, stop=True)
            rb_sb = rbp.tile([128, 512], f32, name="rb_sb", tag="rb")
            nc.vector.tensor_copy(rb_sb[:], rb[:])
            for dm in range(2):
                nc.vector.tensor_mul(ot[:, dm, q0:q0 + 512],
                                     ov[dm][:], rb_sb[:])
        for dm in range(2):
            nc.sync.dma_start(og_ins[h][dm * 128:(dm + 1) * 128, :],
                              ot[:, dm, :])
        if _WITH_COLL:
            nc.gpsimd.collective_compute(
                "AllGather",
                mybir.AluOpType.bypass,
                replica_groups=[[0, 1, 2, 3], [4, 5, 6, 7]],
                ins=[og_ins[h][:].opt()],
                outs=[og_outs[h][:].opt()],
            )
        else:
            for blk in range(MP):
                nc.sync.dma_start(
                    og_outs[h][blk * HD:(blk + 1) * HD, :], og_ins[h][:])

    HEAD_MS = [(2 * h, 2 * h + 1, 8 + 2 * h, 8 + 2 * h + 1)
               for h in range(H_LOC)]

    # ---------------- phase 1 (+ interleaved attention) ----------------
    with tc.tile_pool(name="xrp", bufs=1) as xrp, \
         tc.tile_pool(name="wmp", bufs=2) as wmp, \
         tc.tile_pool(name="sqp", bufs=2) as sqp, \
         tc.tile_pool(name="t1p", bufs=2) as t1p, \
         tc.tile_pool(name="t2p", bufs=2) as t2p, \
         tc.tile_pool(name="trig", bufs=1) as trig:
        cost_sb = trig.tile([ROT, S], f16, name="cost_sb")
        nc.sync.dma_start(cost_sb[:], cost.ap())
        sint_sb = trig.tile([ROT, S], f16, name="sint_sb")
        nc.sync.dma_start(sint_sb[:], sint.ap())

        with tc.tile_pool(name="wvp", bufs=1) as wvp, \
             tc.tile_pool(name="vcp", bufs=3) as vcp:
            # s-slice 0: per-head Q/K through DRAM; V head-pairs
            # interleaved so wvt prefetches ride under QK compute
            xr0 = xrp.tile([128, DT, 1024], f16, name="xr0", tag="xr")
            nc.sync.dma_start(xr0[:], xt2.ap()[:, :, 0:1024])
            for h in range(H_LOC):
                for m in HEAD_MS[h]:
                    emit_qk_tile(m, 0, xr0, wmp, sqp, t1p, t2p,
                                 cost_sb, sint_sb)
                if h < 2:
                    emit_v_vn(0, h, xr0, wvp, vcp)

            # s-slice 1: V(heads 0/1) first, then per head: Q/K direct to
            # SBUF + attention + AllGather; V(heads 2/3) rides after att0
            xr1 = xrp.tile([128, DT, 1024], f16, name="xr1", tag="xr")
            nc.sync.dma_start(xr1[:], xt2.ap()[:, :, 1024:2048])
            emit_v_vn(1, 0, xr1, wvp, vcp)

            with tc.tile_pool(name="qtp", bufs=2) as qtp, \
                 tc.tile_pool(name="ktp", bufs=2) as ktp, \
                 tc.tile_pool(name="vtp", bufs=2) as vtp, \
                 tc.tile_pool(name="etp", bufs=3) as etp, \
                 tc.tile_pool(name="etmp", bufs=1) as etmp, \
                 tc.tile_pool(name="otp", bufs=1) as otp, \
                 tc.tile_pool(name="rbp", bufs=1) as rbp, \
                 tc.tile_pool(name="rip", bufs=2) as rip, \
                 tc.tile_pool(name="mkp", bufs=1) as mkp:
                masks_sb = mkp.tile([128, 4, 512], f16, name="masks_sb")
                nc.sync.dma_start(masks_sb[:], masks.ap())
                pools = (qtp, ktp, vtp, etp, etmp, otp, rbp, rip, masks_sb)
                for h in range(H_LOC):
                    qt = qtp.tile([128, 2, S], f16, name="qt", tag="qt")
                    kt = ktp.tile([128, 2, S], f16, name="kt", tag="kt")
                    qsrc = qT_hs[h][:].rearrange("(dd p) s -> p dd s", p=128)
                    ksrc = kT_hs[h][:].rearrange("(dd p) s -> p dd s", p=128)
                    for dd in range(2):
                        nc.sync.dma_start(qt[:, dd, 0:1024], qsrc[:, dd, :])
                        nc.sync.dma_start(kt[:, dd, 0:1024], ksrc[:, dd, :])
                    for m in HEAD_MS[h]:
                        emit_qk_tile(m, 1, xr1, wmp, sqp, t1p, t2p,
                                     cost_sb, sint_sb,
                                     direct=qt if m < 8 else kt)
                    emit_attention(h, pools, qt, kt)
                    if h == 0:
                        emit_v_vn(1, 1, xr1, wvp, vcp)

    # ---------------- out projection ----------------
    # h is the outer loop so each head's contraction tiles run as soon as
    # its AllGather lands; partial sums accumulate in SBUF f32 tiles.
    og_rs = [og_outs[h][:].rearrange("(j p) s -> p j s", p=128)
             for h in range(H_LOC)]
    with tc.tile_pool(name="wop", bufs=1) as wop, \
         tc.tile_pool(name="omp", bufs=2) as omp, \
         tc.tile_pool(name="accp", bufs=1) as accp, \
         tc.tile_pool(name="accfp", bufs=4) as accfp:
        wo_sb = wop.tile([128, DT, LOCAL], f16, name="wo_sb")
        nc.sync.dma_start(wo_sb[:], wo2.ap())
        accs = [[accp.tile([128, 512], f32, name=f"acc{b}_{cn}")
                 for cn in range(2)] for b in range(16)]
        for h in range(H_LOC):
            om = omp.tile([128, 8, S], f16, name="om", tag="om")
            nc.sync.dma_start(om[:], og_rs[h])
            for sblk in range(4):
                for si in range(4):
                    b = sblk * 4 + si
                    s0 = sblk * 512 + si * 128
                    for cn in range(2):
                        ps3 = psp.tile([128, 512], f32, name="ps3", tag="ps")
                        for j in range(8):
                            nc.tensor.matmul(
                                ps3[:], om[:, j, s0:s0 + 128],
                                wo_sb[:, h * 8 + j, cn * 512:(cn + 1) * 512],
                                start=(j == 0), stop=(j == 7))
                        acc = accs[b][cn]
                        if h == 0:
                            nc.vector.tensor_copy(acc[:], ps3[:])
                        elif h < H_LOC - 1:
                            nc.vector.tensor_add(acc[:], acc[:], ps3[:])
                        else:
                            accf = accfp.tile([128, 512], f16,
                                              name="accf", tag="accf")
                            nc.vector.tensor_add(accf[:], acc[:], ps3[:])
                            nc.sync.dma_start(
                                y.ap()[b * 128:(b + 1) * 128,
                                       cn * 512:(cn + 1) * 512], accf[:])


def _build_program(n_repeat=1):
    import concourse.bass as bass  # noqa: F401
    import concourse.tile as tile
    from concourse import bacc, mybir

    f32 = mybir.dt.float32
    f32r = mybir.dt.float32r
    f16 = mybir.dt.float16

    nc = bacc.Bacc("TRN2", target_bir_lowering=False, debug=False,
                   enable_asserts=True, num_devices=N_CORES)

    xt2 = nc.dram_tensor("xt2", [128, DT, S], f16, kind="ExternalInput")
    wqk = nc.dram_tensor("wqk", [128, 16, DT, 128], f16, kind="ExternalInput")
    wv2 = nc.dram_tensor("wv2", [128, 2, DT, 512], f16, kind="ExternalInput")
    wo2 = nc.dram_tensor("wo2", [128, DT, LOCAL], f16, kind="ExternalInput")
    cost = nc.dram_tensor("cost", [ROT, S], f16, kind="ExternalInput")
    sint = nc.dram_tensor("sint", [ROT, S], f16, kind="ExternalInput")
    rt = nc.dram_tensor("rt", [ROT, ROT], f16, kind="ExternalInput")
    ones = nc.dram_tensor("ones", [128, 1], f16, kind="ExternalInput")
    onesr = nc.dram_tensor("onesr", [1, 128], f32r, kind="ExternalInput")
    masks = nc.dram_tensor("masks", [128, 4, 512], f16, kind="ExternalInput")
    y = nc.dram_tensor("y", [S, LOCAL], f16, kind="ExternalOutput")

    with tile.TileContext(nc) as tc:
        with tc.tile_pool(name="dram", bufs=1, space="DRAM") as dpool, \
             tc.tile_pool(name="const", bufs=1) as cpool, \
             tc.tile_pool(name="psum", bufs=8, space="PSUM") as psp:
            qT_hs = [dpool.tile([HD, 1024], f16, name=f"qT_h{h}")
                     for h in range(H_LOC)]
            kT_hs = [dpool.tile([HD, 1024], f16, name=f"kT_h{h}")
                     for h in range(H_LOC)]
            v_d01 = dpool.tile([128, 2, S // 128, HD], f16, name="v_d01")
            v_d23 = dpool.tile([128, 2, S // 128, HD], f16, name="v_d23")
            og_ins = [dpool.tile([HD, S], f16, name=f"og_in{h}")
                      for h in range(H_LOC)]
            og_outs = [dpool.tile([MP * HD, S], f16, name=f"og_out{h}")
                       for h in range(H_LOC)]

            rt_sb = cpool.tile([ROT, ROT], f16, name="rt_sb")
            nc.sync.dma_start(rt_sb[:], rt.ap())
            ones_sb = cpool.tile([128, 1], f16, name="ones_sb")
            nc.sync.dma_start(ones_sb[:], ones.ap())
            onesr_sb = cpool.tile([1, 128], f32r, name="onesr_sb")
            nc.sync.dma_start(onesr_sb[:], onesr.ap())

            tens = (xt2, wqk, wv2, wo2, cost, sint, masks, y,
                    qT_hs, kT_hs, v_d01, v_d23, og_ins, og_outs,
                    rt_sb, ones_sb, onesr_sb)
            for rep in range(n_repeat):
                _emit_body(nc, tc, tens, psp, rep)

    nc.compile()
    return nc


def _rotary_tables(position_ids):
    """Transposed, interleave-repeated sin/cos tables: [64, S] per batch."""
    pos = np.asarray(position_ids).astype(np.int64)
    inv_freq = 1.0 / (10000.0 ** (np.arange(0, ROT, 2, dtype=np.float32) / ROT))
    sinusoid = np.arange(2048, dtype=np.float32)[:, None] * inv_freq[None, :]
    sin_t = np.sin(sinusoid).astype(np.float32)   # [2048, 32]
    cos_t = np.cos(sinusoid).astype(np.float32)
    outs = []
    for b in range(pos.shape[0]):
        sg = np.repeat(sin_t[pos[b]], 2, axis=1).T   # [64, S]
        cg = np.repeat(cos_t[pos[b]], 2, axis=1).T
        outs.append((np.ascontiguousarray(sg).astype(np.float16),
                     np.ascontiguousarray(cg).astype(np.float16)))
    return outs


def _consts():
    rt_np = np.zeros((ROT, ROT), dtype=np.float16)
    for i in range(ROT // 2):
        rt_np[2 * i + 1, 2 * i] = -1.0   # rt = R^T for rotate_every_two
        rt_np[2 * i, 2 * i + 1] = 1.0
    ones_np = np.ones((128, 1), dtype=np.float16)
    onesr_np = np.ones((1, 128), dtype=np.float32)
    masks_np = np.zeros((128, 4, 512), dtype=np.float16)
    ii = np.arange(128)[:, None]
    qq = np.arange(512)[None, :]
    for j in range(4):
        masks_np[:, j, :] = (128 * j + ii <= qq).astype(np.float16)
    return rt_np, onesr_np, ones_np, masks_np


def _in_maps(hidden_states, position_ids, W_qkv, W_out):
    hs = np.asarray(hidden_states, dtype=np.float32)
    wqkv = np.asarray(W_qkv, dtype=np.float32)
    wout = np.asarray(W_out, dtype=np.float32)
    rt_np, onesr_np, ones_np, masks_np = _consts()
    trig = _rotary_tables(position_ids)

    # x^T pre-swizzled: xt2[p, dt, s] = x[s, dt*128+p]
    xt2s = [np.ascontiguousarray(
                hs[b].T.reshape(DT, 128, S).transpose(1, 0, 2)
            ).astype(np.float16) for b in range(B)]

    in_maps = []
    for c in range(N_CORES):
        dp, tp = c // MP, c % MP
        wl = wqkv[:, tp * 3 * LOCAL:(tp + 1) * 3 * LOCAL]
        wq_ = wl[:, 0:LOCAL]
        wv_ = wl[:, LOCAL:2 * LOCAL]
        wk_ = wl[:, 2 * LOCAL:3 * LOCAL]
        # wqk[p, m, dt, c] = w[dt*128+p, m*128+c], q tiles then k tiles
        wq_r = wq_.reshape(DT, 128, 8, 128).transpose(1, 2, 0, 3)
        wk_r = wk_.reshape(DT, 128, 8, 128).transpose(1, 2, 0, 3)
        wqk_np = np.ascontiguousarray(
            np.concatenate([wq_r, wk_r], axis=1)).astype(np.float16)
        # wv2[p, vn, dt, c] = wv[dt*128+p, vn*512+c]
        wv_np = np.ascontiguousarray(
            wv_.reshape(DT, 128, 2, 512).transpose(1, 2, 0, 3)
        ).astype(np.float16)
        # wo2[p, h*8 + tp_src*2 + dm, c] = wo[tp_src*1024+h*256+dm*128+p, c]
        wo_slice = wout[:, tp * LOCAL:(tp + 1) * LOCAL]
        wo_np = np.ascontiguousarray(
            wo_slice.reshape(MP, H_LOC, 2, 128, LOCAL)
            .transpose(3, 1, 0, 2, 4).reshape(128, DT, LOCAL)
        ).astype(np.float16)
        sg, cg = trig[dp]
        in_maps.append({
            "xt2": xt2s[dp],
            "wqk": wqk_np,
            "wv2": wv_np,
            "wo2": wo_np,
            "cost": cg, "sint": sg,
            "rt": rt_np, "ones": ones_np, "onesr": onesr_np,
            "masks": masks_np,
        })
    return in_maps


def _get_runner(n_repeat=1):
    key = ("runner", n_repeat, _WITH_COLL)
    if key in _CACHE:
        return _CACHE[key]
    import jax
    from jax.sharding import Mesh, PartitionSpec, NamedSharding
    from jax.experimental.shard_map import shard_map
    from concourse import bass2jax, mybir

    nc = _build_program(n_repeat=n_repeat)
    bass2jax.install_neuronx_cc_hook()

    partition_name = (nc.partition_id_tensor.name
                      if nc.partition_id_tensor else None)
    in_names, out_names, out_avals, zero_outs = [], [], [], []
    for alloc in nc.m.functions[0].allocations:
        if not isinstance(alloc, mybir.MemoryLocationSet):
            continue
        name = alloc.memorylocations[0].name
        if alloc.kind == "ExternalInput":
            if name != partition_name:
                in_names.append(name)
        elif alloc.kind == "ExternalOutput":
            shape = tuple(alloc.tensor_shape)
            dtype = mybir.dt.np(alloc.dtype)
            out_names.append(name)
            out_avals.append(jax.core.ShapedArray(shape, dtype))
            zero_outs.append(np.zeros(shape, dtype))
    n_params = len(in_names)
    all_names = in_names + out_names
    if partition_name is not None:
        all_names = all_names + [partition_name]

    def _body(*args):
        operands = list(args)
        if partition_name is not None:
            operands.append(bass2jax.partition_id_tensor())
        outs = bass2jax._bass_exec_p.bind(
            *operands,
            out_avals=tuple(out_avals),
            in_names=tuple(all_names),
            out_names=tuple(out_names),
            lowering_input_output_aliases=(),
            sim_require_finite=True,
            sim_require_nnan=True,
            nc=nc,
        )
        return tuple(outs)

    devices = jax.devices()[:N_CORES]
    mesh = Mesh(np.asarray(devices), ("core",))
    n_outs = len(out_names)
    sharded = jax.jit(
        shard_map(_body, mesh=mesh,
                  in_specs=(PartitionSpec("core"),) * (n_params + n_outs),
                  out_specs=(PartitionSpec("core"),) * n_outs,
                  check_rep=False),
        keep_unused=True,
    )
    sharding = NamedSharding(mesh, PartitionSpec("core"))
    runner = {
        "nc": nc, "sharded": sharded, "in_names": in_names,
        "out_names": out_names, "out_avals": out_avals,
        "zero_outs": zero_outs, "sharding": sharding, "jax": jax,
    }
    _CACHE[key] = runner
    return runner


def _stage(runner, in_maps):
    jax = runner["jax"]
    concat_in = [
        np.concatenate([np.asarray(in_maps[c][name]) for c in range(N_CORES)],
                       axis=0)
        for name in runner["in_names"]
    ]
    concat_zero = [
        np.zeros((N_CORES * z.shape[0], *z.shape[1:]), z.dtype)
        for z in runner["zero_outs"]
    ]
    return [jax.device_put(a, runner["sharding"]) for a in concat_in + concat_zero]


def _execute(runner, staged):
    jax = runner["jax"]
    outs = runner["sharded"](*staged)
    outs = jax.block_until_ready(outs)
    return outs


def kernel(hidden_states, position_ids, W_qkv, W_out):
    runner = _get_runner()
    in_maps = _in_maps(hidden_states, position_ids, W_qkv, W_out)
    staged = _stage(runner, in_maps)
    outs = _execute(runner, staged)
    yc = np.asarray(outs[0]).astype(np.float32).reshape(N_CORES, S, LOCAL)
    result = np.empty((B, S, D), dtype=np.float32)
    for c in range(N_CORES):
        dp, tp = c // MP, c % MP
        result[dp][:, tp * LOCAL:(tp + 1) * LOCAL] = yc[c]
    return result


def bench(inputs, iters=10, n_repeat=1):
    """Return per-call wall-clock seconds (list) for the staged executable."""
    import time
    runner = _get_runner(n_repeat)
    in_maps = _in_maps(**inputs)
    staged = _stage(runner, in_maps)
    _execute(runner, staged)  # warm-up / compile
    times = []
    for _ in range(iters):
        t0 = time.perf_counter()
        _execute(runner, staged)
        times.append(time.perf_counter() - t0)
    return times


def bench_burst(inputs, R, samples=3, n_repeat=1):
    """Wall seconds for R back-to-back dispatches, blocked once at the end.

    The slope over R isolates per-call device execution time from the
    (dominant, fixed) axon round-trip latency.
    """
    import time
    runner = _get_runner(n_repeat)
    in_maps = _in_maps(**inputs)
    staged = _stage(runner, in_maps)
    jax = runner["jax"]
    sharded = runner["sharded"]
    jax.block_until_ready(sharded(*staged))  # warm-up
    times = []
    for _ in range(samples):
        t0 = time.perf_counter()
        outs = None
        for _ in range(R):
            outs = sharded(*staged)
        jax.block_until_ready(outs)
        times.append(time.perf_counter() - t0)
    return times


# revision 23
# speedup vs baseline: 1.0440x; 1.0440x over previous
"""CodeGen attention on 8 Trainium2 NeuronCores (Bass/Tile), fp16 datapath.

Sharding: tensor-parallel over the 4 CodeGen mp head-groups x data-parallel
over batch 2. Core c = dp*4 + tp handles batch dp, head group tp (4 heads).

Per-core pipeline (all matmul inputs fp16, PSUM accumulation fp32):
  QKV (per 1024-wide s slice): V in natural [s, d] layout (x^T stationary,
      Wv moving), then Q^T/K^T (W stationary, x^T moving) with rotary
      fused on the first 64 rows of each head.  Weights arrive host
      pre-swizzled so every DMA runs fat contiguous per-partition
      segments.  The second s-slice's Q/K tiles are emitted head by head,
      interleaved with that head's attention, so the per-head AllGather
      collectives start as early as possible and overlap compute.
  Attention: scores^T tiles, softmax without max-subtraction (logits are
      O(5)), column sums via an ones-vector matmul, PV accumulated in
      PSUM, late normalize by 1/rowsum broadcast through a K=1 matmul.
  Out-projection: contraction tiles stream per head so accumulation
      starts as soon as AllGather h lands.  q/k/og scratch is split into
      per-head DRAM tensors to keep cross-phase dependencies fine-grained.

Host assembles the [B, S, D] output from per-core [S, D/4] column shards.
"""

import numpy as np

B, S, D = 2, 2048, 4096
N_HEAD = 16
HD = 256
MP = 4
ROT = 64
LOCAL = D // MP            # 1024 (= 4 heads * 256)
DT = D // 128              # 32 contraction tiles
N_CORES = 8
H_LOC = N_HEAD // MP       # 4 heads per core

_CACHE = {}
_WITH_COLL = True   # timing-probe flag: False replaces AllGather with DMA copies


def _emit_body(nc, tc, tens, psp, rep):
    """One full pipeline pass (scratch tensors shared across reps)."""
    import concourse.tile as tile  # noqa: F401
    from concourse import mybir

    f32 = mybir.dt.float32
    f32r = mybir.dt.float32r
    f16 = mybir.dt.float16
    EXP = mybir.ActivationFunctionType.Exp

    (xt2, wqk, wv2, wo2, cost, sint, masks, y,
     qT_hs, kT_hs, v_d01, v_d23, og_ins, og_outs,
     rt_sb, ones_sb, onesr_sb) = tens

    def emit_v_vn(ss, vn, xr, wvp, vcp):
        wvt = wvp.tile([128, DT, 512], f16, name="wvt", tag="wvt")
        nc.sync.dma_start(wvt[:], wv2.ap()[:, vn])
        vd = v_d01 if vn == 0 else v_d23
        for sm in range(8):
            pv = psp.tile([128, 512], f32, name="pv", tag="ps")
            for dt in range(DT):
                nc.tensor.matmul(
                    pv[:], xr[:, dt, sm * 128:(sm + 1) * 128],
                    wvt[:, dt, :], start=(dt == 0), stop=(dt == DT - 1))
            vc = vcp.tile([128, 512], f16, name="vc", tag="vc")
            nc.vector.tensor_copy(vc[:], pv[:])
            st = ss * 8 + sm
            for hh in range(2):
                nc.sync.dma_start(
                    vd[:, hh, st, :], vc[:, hh * 256:(hh + 1) * 256])

    def emit_qk_tile(m, ss, xr, wmp, sqp, t1p, t2p, cost_sb, sint_sb,
                     direct=None):
        """direct: a [128, 2, S] SBUF tile — ss1 results land there
        straight from PSUM (no DRAM round-trip)."""
        wm = wmp.tile([128, DT, 128], f16, name="wm", tag="wm")
        nc.sync.dma_start(wm[:], wqk.ap()[:, m])
        dd = m % 2
        for n in range(2):
            ps = psp.tile([128, 512], f32, name="ps", tag="ps")
            for dt in range(DT):
                nc.tensor.matmul(
                    ps[:], wm[:, dt, :], xr[:, dt, n * 512:(n + 1) * 512],
                    start=(dt == 0), stop=(dt == DT - 1))
            sg = ss * 1024 + n * 512
            if direct is None:
                sq = sqp.tile([128, 512], f16, name="sq", tag="sq")
                tgt, tgt_rot = sq[:], sq[0:ROT, :]
            else:
                tgt = direct[:, dd, sg:sg + 512]
                tgt_rot = direct[0:ROT, dd, sg:sg + 512]
            nc.vector.tensor_copy(tgt, ps[:])
            if m % 2 == 0:
                # rows 0:64 are the rotary dims of a head
                rp = psp.tile([128, 512], f32, name="rp", tag="ps")
                nc.tensor.matmul(rp[0:ROT, :], rt_sb[:],
                                 tgt_rot, start=True, stop=True)
                t1 = t1p.tile([ROT, 512], f32, name="t1", tag="t1")
                nc.vector.tensor_mul(t1[:], ps[0:ROT, :],
                                     cost_sb[:, sg:sg + 512])
                t2 = t2p.tile([ROT, 512], f32, name="t2", tag="t2")
                nc.vector.tensor_mul(t2[:], rp[0:ROT, :],
                                     sint_sb[:, sg:sg + 512])
                nc.vector.tensor_add(tgt_rot, t1[:], t2[:])
            if direct is None:
                dest = qT_hs[m // 2] if m < 8 else kT_hs[(m - 8) // 2]
                nc.sync.dma_start(
                    dest[dd * 128:(dd + 1) * 128, sg:sg + 512], tgt)

    def emit_attention(h, pools, qt, kt):
        qtp, ktp, vtp, etp, etmp, otp, rbp, rip, masks_sb = pools
        vt = vtp.tile([128, S // 128, HD], f16, name="vt", tag="vt")
        vd = v_d01 if h < 2 else v_d23
        nc.sync.dma_start(vt[:], vd[:, h % 2])
        ot = otp.tile([128, 2, S], f16, name="ot", tag="ot")

        for qn in range(4):
            nk = (qn + 1) * 4
            q0 = qn * 512
            rs = psp.tile([1, 512], f32, name="rs", tag="ps")
            ov = [psp.tile([128, 512], f32, name=f"ov{dm}", tag="ps")
                  for dm in range(2)]
            for ki in range(nk):
                sp = psp.tile([128, 512], f32, name="sp", tag="ps")
                for dd in range(2):
                    nc.tensor.matmul(
                        sp[:], kt[:, dd, ki * 128:(ki + 1) * 128],
                        qt[:, dd, q0:q0 + 512],
                        start=(dd == 0), stop=(dd == 1))
                et = etp.tile([128, 512], f16, name="et", tag="et")
                if ki >= qn * 4:
                    etm = etmp.tile([128, 512], f16, name="etm", tag="etm")
                    nc.scalar.activation(etm[:], sp[:], EXP,
                                         bias=0.0, scale=1.0 / 16.0)
                    nc.vector.tensor_mul(et[:], etm[:],
                                         masks_sb[:, ki - qn * 4, :])
                else:
                    nc.scalar.activation(et[:], sp[:], EXP,
                                         bias=0.0, scale=1.0 / 16.0)
                nc.tensor.matmul(rs[:], ones_sb[:], et[:],
                                 start=(ki == 0), stop=(ki == nk - 1))
                for dm in range(2):
                    nc.tensor.matmul(
                        ov[dm][:], vt[:, ki, dm * 128:(dm + 1) * 128],
                        et[:], start=(ki == 0), stop=(ki == nk - 1))
            rinv = rip.tile([1, 512], f32r, name="rinv", tag="rinv")
            # f32r is bit-identical to f32 here; only the matmul
            # datapath reads it differently.
            with nc.allow_low_precision(reason="f32r == f32 bits"):
                nc.vector.reciprocal(rinv[:], rs[:])
            rb = psp.tile([128, 512], f32, name="rb", tag="ps")
            nc.tensor.matmul(rb[:], onesr_sb[:], rinv[:],
                             start=True, stop=True)
            rb_sb = rbp.tile([128, 512], f32, name="rb_sb", tag="rb")
            nc.vector.tensor_copy(rb_sb[:], rb[:])
            for dm in range(2):
                nc.vector.tensor_mul(ot[:, dm, q0:q0 + 512],
                                     ov[dm][:], rb_sb[:])
        for dm in range(2):
            nc.sync.dma_start(og_ins[h][dm * 128:(dm + 1) * 128, :],
                              ot[:, dm, :])
        if _WITH_COLL:
            nc.gpsimd.collective_compute(
                "AllGather",
                mybir.AluOpType.bypass,
                replica_groups=[[0, 1, 2, 3], [4, 5, 6, 7]],
                ins=[og_ins[h][:].opt()],
                outs=[og_outs[h][:].opt()],
            )
        else:
            for blk in range(MP):
                nc.sync.dma_start(
                    og_outs[h][blk * HD:(blk + 1) * HD, :], og_ins[h][:])

    HEAD_MS = [(2 * h, 2 * h + 1, 8 + 2 * h, 8 + 2 * h + 1)
               for h in range(H_LOC)]

    # ---------------- phase 1 (+ interleaved attention) ----------------
    with tc.tile_pool(name="xrp", bufs=1) as xrp, \
         tc.tile_pool(name="wmp", bufs=2) as wmp, \
         tc.tile_pool(name="sqp", bufs=2) as sqp, \
         tc.tile_pool(name="t1p", bufs=2) as t1p, \
         tc.tile_pool(name="t2p", bufs=2) as t2p, \
         tc.tile_pool(name="trig", bufs=1) as trig:
        cost_sb = trig.tile([ROT, S], f16, name="cost_sb")
        nc.sync.dma_start(cost_sb[:], cost.ap())
        sint_sb = trig.tile([ROT, S], f16, name="sint_sb")
        nc.sync.dma_start(sint_sb[:], sint.ap())

        with tc.tile_pool(name="wvp", bufs=1) as wvp, \
             tc.tile_pool(name="vcp", bufs=3) as vcp:
            # s-slice 0: per-head Q/K through DRAM; V head-pairs
            # interleaved so wvt prefetches ride under QK compute
            xr0 = xrp.tile([128, DT, 1024], f16, name="xr0", tag="xr")
            nc.sync.dma_start(xr0[:], xt2.ap()[:, :, 0:1024])
            for h in range(H_LOC):
                for m in HEAD_MS[h]:
                    emit_qk_tile(m, 0, xr0, wmp, sqp, t1p, t2p,
                                 cost_sb, sint_sb)
                if h < 2:
                    emit_v_vn(0, h, xr0, wvp, vcp)

            # s-slice 1: V(heads 0/1) first, then per head: Q/K direct to
            # SBUF + attention + AllGather; V(heads 2/3) rides after att0
            xr1 = xrp.tile([128, DT, 1024], f16, name="xr1", tag="xr")
            nc.sync.dma_start(xr1[:], xt2.ap()[:, :, 1024:2048])
            emit_v_vn(1, 0, xr1, wvp, vcp)

            with tc.tile_pool(name="qtp", bufs=2) as qtp, \
                 tc.tile_pool(name="ktp", bufs=2) as ktp, \
                 tc.tile_pool(name="vtp", bufs=2) as vtp, \
                 tc.tile_pool(name="etp", bufs=3) as etp, \
                 tc.tile_pool(name="etmp", bufs=1) as etmp, \
                 tc.tile_pool(name="otp", bufs=1) as otp, \
                 tc.tile_pool(name="rbp", bufs=1) as rbp, \
                 tc.tile_pool(name="rip", bufs=2) as rip, \
                 tc.tile_pool(name="mkp", bufs=1) as mkp:
                masks_sb = mkp.tile([128, 4, 512], f16, name="masks_sb")
                nc.sync.dma_start(masks_sb[:], masks.ap())
                pools = (qtp, ktp, vtp, etp, etmp, otp, rbp, rip, masks_sb)
                for h in range(H_LOC):
                    qt = qtp.tile([128, 2, S], f16, name="qt", tag="qt")
                    kt = ktp.tile([128, 2, S], f16, name="kt", tag="kt")
                    qsrc = qT_hs[h][:].rearrange("(dd p) s -> p dd s", p=128)
                    ksrc = kT_hs[h][:].rearrange("(dd p) s -> p dd s", p=128)
                    for dd in range(2):
                        nc.sync.dma_start(qt[:, dd, 0:1024], qsrc[:, dd, :])
                        nc.sync.dma_start(kt[:, dd, 0:1024], ksrc[:, dd, :])
                    for m in HEAD_MS[h]:
                        emit_qk_tile(m, 1, xr1, wmp, sqp, t1p, t2p,
                                     cost_sb, sint_sb,
                                     direct=qt if m < 8 else kt)
                    emit_attention(h, pools, qt, kt)
                    if h == 0:
                        emit_v_vn(1, 1, xr1, wvp, vcp)

    # ---------------- out projection ----------------
    # h is the outer loop so each head's contraction tiles run as soon as
    # its AllGather lands; partial sums accumulate in SBUF f32 tiles.
    og_rs = [og_outs[h][:].rearrange("(j p) s -> p j s", p=128)
             for h in range(H_LOC)]
    with tc.tile_pool(name="wop", bufs=1) as wop, \
         tc.tile_pool(name="omp", bufs=3) as omp, \
         tc.tile_pool(name="accp", bufs=1) as accp, \
         tc.tile_pool(name="accfp", bufs=4) as accfp:
        wo_sb = wop.tile([128, DT, LOCAL], f16, name="wo_sb")
        nc.sync.dma_start(wo_sb[:], wo2.ap())
        accs = [[accp.tile([128, 512], f32, name=f"acc{b}_{cn}")
                 for cn in range(2)] for b in range(16)]
        for h in range(H_LOC):
            for sblk in range(4):
                om = omp.tile([128, 8, 512], f16, name="om", tag="om")
                nc.sync.dma_start(
                    om[:], og_rs[h][:, :, sblk * 512:(sblk + 1) * 512])
                for si in range(4):
                    b = sblk * 4 + si
                    for cn in range(2):
                        ps3 = psp.tile([128, 512], f32, name="ps3", tag="ps")
                        for j in range(8):
                            nc.tensor.matmul(
                                ps3[:], om[:, j, si * 128:(si + 1) * 128],
                                wo_sb[:, h * 8 + j, cn * 512:(cn + 1) * 512],
                                start=(j == 0), stop=(j == 7))
                        acc = accs[b][cn]
                        if h == 0:
                            nc.vector.tensor_copy(acc[:], ps3[:])
                        elif h < H_LOC - 1:
                            nc.vector.tensor_add(acc[:], acc[:], ps3[:])
                        else:
                            accf = accfp.tile([128, 512], f16,
                                              name="accf", tag="accf")
                            nc.vector.tensor_add(accf[:], acc[:], ps3[:])
                            nc.sync.dma_start(
                                y.ap()[b * 128:(b + 1) * 128,
                                       cn * 512:(cn + 1) * 512], accf[:])


def _build_program(n_repeat=1):
    import concourse.bass as bass  # noqa: F401
    import concourse.tile as tile
    from concourse import bacc, mybir

    f32 = mybir.dt.float32
    f32r = mybir.dt.float32r
    f16 = mybir.dt.float16

    nc = bacc.Bacc("TRN2", target_bir_lowering=False, debug=False,
                   enable_asserts=True, num_devices=N_CORES)

    xt2 = nc.dram_tensor("xt2", [128, DT, S], f16, kind="ExternalInput")
    wqk = nc.dram_tensor("wqk", [128, 16, DT, 128], f16, kind="ExternalInput")
    wv2 = nc.dram_tensor("wv2", [128, 2, DT, 512], f16, kind="ExternalInput")
    wo2 = nc.dram_tensor("wo2", [128, DT, LOCAL], f16, kind="ExternalInput")
    cost = nc.dram_tensor("cost", [ROT, S], f16, kind="ExternalInput")
    sint = nc.dram_tensor("sint", [ROT, S], f16, kind="ExternalInput")
    rt = nc.dram_tensor("rt", [ROT, ROT], f16, kind="ExternalInput")
    ones = nc.dram_tensor("ones", [128, 1], f16, kind="ExternalInput")
    onesr = nc.dram_tensor("onesr", [1, 128], f32r, kind="ExternalInput")
    masks = nc.dram_tensor("masks", [128, 4, 512], f16, kind="ExternalInput")
    y = nc.dram_tensor("y", [S, LOCAL], f16, kind="ExternalOutput")

    with tile.TileContext(nc) as tc:
        with tc.tile_pool(name="dram", bufs=1, space="DRAM") as dpool, \
             tc.tile_pool(name="const", bufs=1) as cpool, \
             tc.tile_pool(name="psum", bufs=8, space="PSUM") as psp:
            qT_hs = [dpool.tile([HD, 1024], f16, name=f"qT_h{h}")
                     for h in range(H_LOC)]
            kT_hs = [dpool.tile([HD, 1024], f16, name=f"kT_h{h}")
                     for h in range(H_LOC)]
            v_d01 = dpool.tile([128, 2, S // 128, HD], f16, name="v_d01")
            v_d23 = dpool.tile([128, 2, S // 128, HD], f16, name="v_d23")
            og_ins = [dpool.tile([HD, S], f16, name=f"og_in{h}")
                      for h in range(H_LOC)]
            og_outs = [dpool.tile([MP * HD, S], f16, name=f"og_out{h}")
                       for h in range(H_LOC)]

            rt_sb = cpool.tile([ROT, ROT], f16, name="rt_sb")
            nc.sync.dma_start(rt_sb[:], rt.ap())
            ones_sb = cpool.tile([128, 1], f16, name="ones_sb")
            nc.sync.dma_start(ones_sb[:], ones.ap())
            onesr_sb = cpool.tile([1, 128], f32r, name="onesr_sb")
            nc.sync.dma_start(onesr_sb[:], onesr.ap())

            tens = (xt2, wqk, wv2, wo2, cost, sint, masks, y,
                    qT_hs, kT_hs, v_d01, v_d23, og_ins, og_outs,
                    rt_sb, ones_sb, onesr_sb)
            for rep in range(n_repeat):
                _emit_body(nc, tc, tens, psp, rep)

    nc.compile()
    return nc


def _rotary_tables(position_ids):
    """Transposed, interleave-repeated sin/cos tables: [64, S] per batch."""
    pos = np.asarray(position_ids).astype(np.int64)
    inv_freq = 1.0 / (10000.0 ** (np.arange(0, ROT, 2, dtype=np.float32) / ROT))
    sinusoid = np.arange(2048, dtype=np.float32)[:, None] * inv_freq[None, :]
    sin_t = np.sin(sinusoid).astype(np.float32)   # [2048, 32]
    cos_t = np.cos(sinusoid).astype(np.float32)
    outs = []
    for b in range(pos.shape[0]):
        sg = np.repeat(sin_t[pos[b]], 2, axis=1).T   # [64, S]
        cg = np.repeat(cos_t[pos[b]], 2, axis=1).T
        outs.append((np.ascontiguousarray(sg).astype(np.float16),
                     np.ascontiguousarray(cg).astype(np.float16)))
    return outs


def _consts():
    rt_np = np.zeros((ROT, ROT), dtype=np.float16)
    for i in range(ROT // 2):
        rt_np[2 * i + 1, 2 * i] = -1.0   # rt = R^T for rotate_every_two
        rt_np[2 * i, 2 * i + 1] = 1.0
    ones_np = np.ones((128, 1), dtype=np.float16)
    onesr_np = np.ones((1, 128), dtype=np.float32)
    masks_np = np.zeros((128, 4, 512), dtype=np.float16)
    ii = np.arange(128)[:, None]
    qq = np.arange(512)[None, :]
    for j in range(4):
        masks_np[:, j, :] = (128 * j + ii <= qq).astype(np.float16)
    return rt_np, onesr_np, ones_np, masks_np


def _in_maps(hidden_states, position_ids, W_qkv, W_out):
    hs = np.asarray(hidden_states, dtype=np.float32)
    wqkv = np.asarray(W_qkv, dtype=np.float32)
    wout = np.asarray(W_out, dtype=np.float32)
    rt_np, onesr_np, ones_np, masks_np = _consts()
    trig = _rotary_tables(position_ids)

    # x^T pre-swizzled: xt2[p, dt, s] = x[s, dt*128+p]
    xt2s = [np.ascontiguousarray(
                hs[b].T.reshape(DT, 128, S).transpose(1, 0, 2)
            ).astype(np.float16) for b in range(B)]

    in_maps = []
    for c in range(N_CORES):
        dp, tp = c // MP, c % MP
        wl = wqkv[:, tp * 3 * LOCAL:(tp + 1) * 3 * LOCAL]
        wq_ = wl[:, 0:LOCAL]
        wv_ = wl[:, LOCAL:2 * LOCAL]
        wk_ = wl[:, 2 * LOCAL:3 * LOCAL]
        # wqk[p, m, dt, c] = w[dt*128+p, m*128+c], q tiles then k tiles
        wq_r = wq_.reshape(DT, 128, 8, 128).transpose(1, 2, 0, 3)
        wk_r = wk_.reshape(DT, 128, 8, 128).transpose(1, 2, 0, 3)
        wqk_np = np.ascontiguousarray(
            np.concatenate([wq_r, wk_r], axis=1)).astype(np.float16)
        # wv2[p, vn, dt, c] = wv[dt*128+p, vn*512+c]
        wv_np = np.ascontiguousarray(
            wv_.reshape(DT, 128, 2, 512).transpose(1, 2, 0, 3)
        ).astype(np.float16)
        # wo2[p, h*8 + tp_src*2 + dm, c] = wo[tp_src*1024+h*256+dm*128+p, c]
        wo_slice = wout[:, tp * LOCAL:(tp + 1) * LOCAL]
        wo_np = np.ascontiguousarray(
            wo_slice.reshape(MP, H_LOC, 2, 128, LOCAL)
            .transpose(3, 1, 0, 2, 4).reshape(128, DT, LOCAL)
        ).astype(np.float16)
        sg, cg = trig[dp]
        in_maps.append({
            "xt2": xt2s[dp],
            "wqk": wqk_np,
            "wv2": wv_np,
            "wo2": wo_np,
            "cost": cg, "sint": sg,
            "rt": rt_np, "ones": ones_np, "onesr": onesr_np,
            "masks": masks_np,
        })
    return in_maps


def _get_runner(n_repeat=1):
    key = ("runner", n_repeat, _WITH_COLL)
    if key in _CACHE:
        return _CACHE[key]
    import jax
    from jax.sharding import Mesh, PartitionSpec, NamedSharding
    from jax.experimental.shard_map import shard_map
    from concourse import bass2jax, mybir

    nc = _build_program(n_repeat=n_repeat)
    bass2jax.install_neuronx_cc_hook()

    partition_name = (nc.partition_id_tensor.name
                      if nc.partition_id_tensor else None)
    in_names, out_names, out_avals, zero_outs = [], [], [], []
    for alloc in nc.m.functions[0].allocations:
        if not isinstance(alloc, mybir.MemoryLocationSet):
            continue
        name = alloc.memorylocations[0].name
        if alloc.kind == "ExternalInput":
            if name != partition_name:
                in_names.append(name)
        elif alloc.kind == "ExternalOutput":
            shape = tuple(alloc.tensor_shape)
            dtype = mybir.dt.np(alloc.dtype)
            out_names.append(name)
            out_avals.append(jax.core.ShapedArray(shape, dtype))
            zero_outs.append(np.zeros(shape, dtype))
    n_params = len(in_names)
    all_names = in_names + out_names
    if partition_name is not None:
        all_names = all_names + [partition_name]

    def _body(*args):
        operands = list(args)
        if partition_name is not None:
            operands.append(bass2jax.partition_id_tensor())
        outs = bass2jax._bass_exec_p.bind(
            *operands,
            out_avals=tuple(out_avals),
            in_names=tuple(all_names),
            out_names=tuple(out_names),
            lowering_input_output_aliases=(),
            sim_require_finite=True,
            sim_require_nnan=True,
            nc=nc,
        )
        return tuple(outs)

    devices = jax.devices()[:N_CORES]
    mesh = Mesh(np.asarray(devices), ("core",))
    n_outs = len(out_names)
    sharded = jax.jit(
        shard_map(_body, mesh=mesh,
                  in_specs=(PartitionSpec("core"),) * (n_params + n_outs),
                  out_specs=(PartitionSpec("core"),) * n_outs,
                  check_rep=False),
        keep_unused=True,
    )
    sharding = NamedSharding(mesh, PartitionSpec("core"))
    runner = {
        "nc": nc, "sharded": sharded, "in_names": in_names,
        "out_names": out_names, "out_avals": out_avals,
        "zero_outs": zero_outs, "sharding": sharding, "jax": jax,
    }
    _CACHE[key] = runner
    return runner


def _stage(runner, in_maps):
    jax = runner["jax"]
    concat_in = [
        np.concatenate([np.asarray(in_maps[c][name]) for c in range(N_CORES)],
                       axis=0)
        for name in runner["in_names"]
    ]
    concat_zero = [
        np.zeros((N_CORES * z.shape[0], *z.shape[1:]), z.dtype)
        for z in runner["zero_outs"]
    ]
    return [jax.device_put(a, runner["sharding"]) for a in concat_in + concat_zero]


def _execute(runner, staged):
    jax = runner["jax"]
    outs = runner["sharded"](*staged)
    outs = jax.block_until_ready(outs)
    return outs


def kernel(hidden_states, position_ids, W_qkv, W_out):
    runner = _get_runner()
    in_maps = _in_maps(hidden_states, position_ids, W_qkv, W_out)
    staged = _stage(runner, in_maps)
    outs = _execute(runner, staged)
    yc = np.asarray(outs[0]).astype(np.float32).reshape(N_CORES, S, LOCAL)
    result = np.empty((B, S, D), dtype=np.float32)
    for c in range(N_CORES):
        dp, tp = c // MP, c % MP
        result[dp][:, tp * LOCAL:(tp + 1) * LOCAL] = yc[c]
    return result


def bench(inputs, iters=10, n_repeat=1):
    """Return per-call wall-clock seconds (list) for the staged executable."""
    import time
    runner = _get_runner(n_repeat)
    in_maps = _in_maps(**inputs)
    staged = _stage(runner, in_maps)
    _execute(runner, staged)  # warm-up / compile
    times = []
    for _ in range(iters):
        t0 = time.perf_counter()
        _execute(runner, staged)
        times.append(time.perf_counter() - t0)
    return times


def bench_burst(inputs, R, samples=3, n_repeat=1):
    """Wall seconds for R back-to-back dispatches, blocked once at the end.

    The slope over R isolates per-call device execution time from the
    (dominant, fixed) axon round-trip latency.
    """
    import time
    runner = _get_runner(n_repeat)
    in_maps = _in_maps(**inputs)
    staged = _stage(runner, in_maps)
    jax = runner["jax"]
    sharded = runner["sharded"]
    jax.block_until_ready(sharded(*staged))  # warm-up
    times = []
    for _ in range(samples):
        t0 = time.perf_counter()
        outs = None
        for _ in range(R):
            outs = sharded(*staged)
        jax.block_until_ready(outs)
        times.append(time.perf_counter() - t0)
    return times
